# revision 1
# baseline (speedup 1.0000x reference)
"""Self-contained Trainium2 Bass kernel for the DNAConv GNN message-passing problem.

kernel(**inputs) takes the FULL unsharded inputs and returns the FULL [50000, 64]
float32 output. Edges are sharded across 8 NeuronCores by destination-node range
(6250 nodes/core), so each core owns its output rows and no collectives are needed.
Per core: history rows are fetched per edge with SWDGE transpose-gathers (fp16),
K/V are computed per edge on the PE (fp16 matmuls), token softmax + edge softmax
are fused into an exp-normalize form on DVE/ACT, and per-destination aggregation
is a one-hot segment-sum matmul accumulated in PSUM (128-node blocks).
"""
"""GNN DNAConv kernel for TRN2: builder + host prep.

Per-core (dst-sharded) algorithm:
  out[n] = (Num[n]/Den[n] + bv) @ Wo.T + bo + current[n]
  Num[n,h,d] = sum_{e->n} sum_l w[e,l,h] * v[e,l,h,d]
  Den[n,h]   = sum_{e->n} m[e,h]
  w[e,l,h] = u[e,l,h] * m[e,h] / U[e,h];  u = exp(s), U = sum_l u, m = max_l u
  s[e,l,h] = sum_d q[dst,h,d]*k[src,l,h,d] / 4   (the 1/4 is folded into q)
  (bk provably has no effect; bq/bv/bo handled at node level.)
"""
import numpy as np

import concourse.bacc as bacc
import concourse.tile as tile
from concourse import bass, mybir
from concourse.masks import make_identity

FP16 = mybir.dt.float16
F32 = mybir.dt.float32
I16 = mybir.dt.int16


def wrap16_rep(idx):
    """SWDGE idx layout: [128, n/16], elem j at [j%16, j//16], replicated x8."""
    idx = np.asarray(idx, np.int16)
    n = idx.shape[0]
    assert n % 16 == 0
    w = idx.reshape(n // 16, 16).T
    return np.tile(w, (8, 1)).copy()


def host_prep(inputs, ncores=8):
    hist = np.asarray(inputs["history"], np.float32)
    ei = np.asarray(inputs["edge_index"])
    n_src, L, C = hist.shape
    Wq = np.asarray(inputs["Wq"], np.float32); bq = np.asarray(inputs["bq"], np.float32)
    Wk = np.asarray(inputs["Wk"], np.float32)
    Wv = np.asarray(inputs["Wv"], np.float32); bv = np.asarray(inputs["bv"], np.float32)
    Wo = np.asarray(inputs["Wo"], np.float32); bo = np.asarray(inputs["bo"], np.float32)
    row, col = ei[0].astype(np.int64), ei[1].astype(np.int64)

    nodes_per_core = (n_src + ncores - 1) // ncores
    nblk = (nodes_per_core + 127) // 128
    nloc = nblk * 128
    src_split = ((n_src + 1) // 2 + 127) // 128 * 128
    if src_split >= n_src:
        src_split = n_src // 2

    hist_sw = np.zeros((n_src, 256), np.float16)
    hist_sw[:, :L * C] = hist.reshape(n_src, L * C).astype(np.float16)

    order = np.argsort(col, kind="stable")
    row_s, col_s = row[order], col[order]
    core_of = col_s // nodes_per_core
    loc = col_s % nodes_per_core
    blk = loc // 128
    lo_side = row_s < src_split

    cnt = np.zeros((ncores, nblk, 2), np.int64)
    for c in range(ncores):
        mc = core_of == c
        bb = blk[mc]
        ll = lo_side[mc]
        for b in range(nblk):
            mb = bb == b
            cnt[c, b, 0] = (mb & ll).sum()
            cnt[c, b, 1] = (mb & ~ll).sum()
    T_lo = max(1, int(np.ceil(cnt[:, :, 0].max() / 128)))
    T_hi = max(1, int(np.ceil(cnt[:, :, 1].max() / 128)))
    T_b = T_lo + T_hi
    CH = 6  # tiles per gather chunk (<= 1000 descriptor HW limit / 128)

    WkvT1 = np.concatenate([Wk.T, Wv.T], axis=1).astype(np.float16)
    WkvT = np.concatenate([WkvT1, WkvT1], axis=0)  # duplicated in both partition halves
    WqT = (Wq.T * (1.0 / np.sqrt(C // 4))).astype(np.float32)
    WoT = Wo.T.astype(np.float32)
    bq_rep = np.tile(bq * (1.0 / np.sqrt(C // 4)), (128, 1)).astype(np.float32)
    bv_rep = np.tile(bv, (128, 1)).astype(np.float32)
    bo_rep = np.tile(bo, (128, 1)).astype(np.float32)
    iota128 = np.tile(np.arange(128, dtype=np.float16), (128, 1))
    cur = hist[:, -1]

    in_maps = []
    for c in range(ncores):
        mc = core_of == c
        hist_idx_lo = np.zeros((nblk, 128, T_lo * 8), np.int16)
        hist_idx_hi = np.zeros((nblk, 128, T_hi * 8), np.int16)
        q_idx = np.zeros((nblk, 128, T_b * 8), np.int16)
        dst_rel = np.full((128, nblk * T_b), -1.0, np.float32)
        for b in range(nblk):
            mb = mc & (blk == b)
            r_lo, d_lo = row_s[mb & lo_side], loc[mb & lo_side]
            r_hi, d_hi = row_s[mb & ~lo_side], loc[mb & ~lo_side]
            n_lo, n_hi = len(r_lo), len(r_hi)
            assert n_lo <= T_lo * 128 and n_hi <= T_hi * 128
            slo = np.zeros(T_lo * 128, np.int64); slo[:n_lo] = r_lo
            shi = np.zeros(T_hi * 128, np.int64); shi[:n_hi] = r_hi - src_split
            hist_idx_lo[b] = wrap16_rep(slo)
            hist_idx_hi[b] = wrap16_rep(shi)
            qi = np.full(T_b * 128, 127, np.int64)
            dr = np.full(T_b * 128, -1.0, np.float32)
            qi[:n_lo] = d_lo % 128; dr[:n_lo] = d_lo % 128
            qi[T_lo * 128:T_lo * 128 + n_hi] = d_hi % 128
            dr[T_lo * 128:T_lo * 128 + n_hi] = d_hi % 128
            q_idx[b] = wrap16_rep(qi)
            dst_rel[:, b * T_b:(b + 1) * T_b] = dr.reshape(T_b, 128).T
        cur_c = np.zeros((nloc, C), np.float32)
        nreal = min(nodes_per_core, n_src - c * nodes_per_core)
        cur_c[:nreal] = cur[c * nodes_per_core:c * nodes_per_core + nreal]
        in_maps.append({
            "hist_sw": hist_sw,
            "cur": cur_c,
            "hist_idx_lo": hist_idx_lo, "hist_idx_hi": hist_idx_hi,
            "q_idx": q_idx, "dst_rel": dst_rel,
            "WkvT": np.asarray(WkvT), "WqT": WqT, "WoT": WoT,
            "bq_rep": bq_rep, "bv_rep": bv_rep, "bo_rep": bo_rep,
            "iota128": iota128,
        })
    params = dict(T_lo=T_lo, T_hi=T_hi, n_blocks=nblk, n_src=n_src, nloc=nloc,
                  src_split=src_split, nodes_per_core=nodes_per_core, ncores=ncores)
    return in_maps, params


def build(params, stage=99, reps=1, ablate=()):
    T_lo, T_hi = params["T_lo"], params["T_hi"]
    T_b = T_lo + T_hi
    NB = params["n_blocks"]
    NSRC = params["n_src"]
    NLOC = params["nloc"]
    SPLIT = params["src_split"]

    nc = bacc.Bacc(None, target_bir_lowering=False)
    hist_sw = nc.declare_dram_parameter("hist_sw", [NSRC, 256], FP16, isOutput=False)
    cur = nc.declare_dram_parameter("cur", [NLOC, 64], F32, isOutput=False)
    hidx_lo = nc.declare_dram_parameter("hist_idx_lo", [NB, 128, T_lo * 8], I16, isOutput=False)
    hidx_hi = nc.declare_dram_parameter("hist_idx_hi", [NB, 128, T_hi * 8], I16, isOutput=False)
    qidx = nc.declare_dram_parameter("q_idx", [NB, 128, T_b * 8], I16, isOutput=False)
    dst_rel_d = nc.declare_dram_parameter("dst_rel", [128, NB * T_b], F32, isOutput=False)
    WkvT_d = nc.declare_dram_parameter("WkvT", [128, 128], FP16, isOutput=False)
    WqT_d = nc.declare_dram_parameter("WqT", [64, 64], F32, isOutput=False)
    WoT_d = nc.declare_dram_parameter("WoT", [64, 64], F32, isOutput=False)
    bq_rep_d = nc.declare_dram_parameter("bq_rep", [128, 64], F32, isOutput=False)
    bv_rep_d = nc.declare_dram_parameter("bv_rep", [128, 64], F32, isOutput=False)
    bo_rep_d = nc.declare_dram_parameter("bo_rep", [128, 64], F32, isOutput=False)
    iota_d = nc.declare_dram_parameter("iota128", [128, 128], FP16, isOutput=False)
    out_d = nc.declare_dram_parameter("out", [NLOC, 64], F32, isOutput=True)
    q_dram = nc.dram_tensor("q_table", [NLOC, 64], F32)

    with tile.TileContext(nc) as tc:
        with (
            tc.tile_pool(name="const", bufs=1) as cpool,
            tc.tile_pool(name="meta", bufs=1) as mpool,
            tc.tile_pool(name="idxp", bufs=2) as ipool,
            tc.tile_pool(name="work", bufs=2) as wpool,
            tc.tile_pool(name="small", bufs=3) as spool,
            tc.tile_pool(name="psum", bufs=1, space="PSUM") as ppool,
            tc.tile_pool(name="aggpsum", bufs=1, space="PSUM") as apool,
            tc.tile_pool(name="kvpsum", bufs=2, space="PSUM") as kvpool,
        ):
            WkvT = cpool.tile([128, 128], FP16)
            nc.sync.dma_start(out=WkvT[:], in_=WkvT_d[:])
            WqT = cpool.tile([64, 64], F32)
            nc.sync.dma_start(out=WqT[:], in_=WqT_d[:])
            WoT = cpool.tile([64, 64], F32)
            nc.sync.dma_start(out=WoT[:], in_=WoT_d[:])
            bq_rep = cpool.tile([128, 64], F32)
            nc.sync.dma_start(out=bq_rep[:], in_=bq_rep_d[:])
            bv_rep = cpool.tile([128, 64], F32)
            nc.sync.dma_start(out=bv_rep[:], in_=bv_rep_d[:])
            bo_rep = cpool.tile([128, 64], F32)
            nc.sync.dma_start(out=bo_rep[:], in_=bo_rep_d[:])
            iota = cpool.tile([128, 128], FP16)
            nc.sync.dma_start(out=iota[:], in_=iota_d[:])
            ident = cpool.tile([128, 128], F32)
            make_identity(nc, ident[:])
            dst_rel = mpool.tile([128, NB * T_b], F32)
            nc.sync.dma_start(out=dst_rel[:], in_=dst_rel_d[:])

            for _rep in range(reps):
              # ---- Phase 1: q table: q = (cur @ Wq.T + bq)/sqrt(D) ----
              for b in range(NB):
                  cur_blk = spool.tile([128, 64], F32, tag="curblk")
                  nc.sync.dma_start(out=cur_blk[:], in_=cur[b * 128:(b + 1) * 128, :])
                  curT_p = ppool.tile([64, 128], F32, space="PSUM", tag="misc")
                  nc.tensor.transpose(out=curT_p[:], in_=cur_blk[:], identity=ident[:])
                  curT = spool.tile([64, 128], F32, tag="curT")
                  nc.vector.tensor_copy(out=curT[:], in_=curT_p[:])
                  q_p = ppool.tile([128, 64], F32, space="PSUM", tag="misc", name="q_p")
                  nc.tensor.matmul(q_p[:], lhsT=curT[:], rhs=WqT[:], start=True, stop=True)
                  q_sb = spool.tile([128, 64], F32, tag="qsb")
                  nc.vector.tensor_tensor(out=q_sb[:], in0=q_p[:], in1=bq_rep[:],
                                          op=mybir.AluOpType.add)
                  nc.sync.dma_start(out=q_dram[b * 128:(b + 1) * 128, :], in_=q_sb[:])
                  if stage <= 1:
                      nc.sync.dma_start(out=out_d[b * 128:(b + 1) * 128, :], in_=q_sb[:])

              # ---- Phase 2: per-block edge processing ----
              for b in range(NB if stage >= 2 else 0):
                  il = ipool.tile([128, T_lo * 8], I16, tag="il")
                  nc.sync.dma_start(out=il[:], in_=hidx_lo[b])
                  ih = ipool.tile([128, T_hi * 8], I16, tag="ih")
                  nc.sync.dma_start(out=ih[:], in_=hidx_hi[b])
                  iq = ipool.tile([128, T_b * 8], I16, tag="iq")
                  nc.sync.dma_start(out=iq[:], in_=qidx[b])

                  CH = 6
                  hist_lo = []
                  for c in range(0, T_lo, CH):
                      n_t = min(CH, T_lo - c)
                      hc = wpool.tile([128, 2, n_t * 128], FP16, tag=f"histlo{c}",
                                      name=f"histlo_{b}_{c}")
                      nc.gpsimd.dma_gather(
                          out_ap=hc[:], in_ap=hist_sw[0:SPLIT, :],
                          idxs_ap=il[:, c * 8:(c + n_t) * 8],
                          num_idxs=n_t * 128, num_idxs_reg=n_t * 128,
                          elem_size=256, transpose=True)
                      hist_lo.append(hc)
                  hist_hi = []
                  for c in range(0, T_hi, CH):
                      n_t = min(CH, T_hi - c)
                      hc = wpool.tile([128, 2, n_t * 128], FP16, tag=f"histhi{c}",
                                      name=f"histhi_{b}_{c}")
                      nc.gpsimd.dma_gather(
                          out_ap=hc[:], in_ap=hist_sw[SPLIT:NSRC, :],
                          idxs_ap=ih[:, c * 8:(c + n_t) * 8],
                          num_idxs=n_t * 128, num_idxs_reg=n_t * 128,
                          elem_size=256, transpose=True)
                      hist_hi.append(hc)
                  q_g = wpool.tile([128, T_b, 64], F32, tag="qg")
                  for c in range(0, T_b, CH):
                      n_t = min(CH, T_b - c)
                      nc.gpsimd.dma_gather(
                          out_ap=q_g[:, c:c + n_t, :],
                          in_ap=q_dram[b * 128:(b + 1) * 128, :],
                          idxs_ap=iq[:, c * 8:(c + n_t) * 8],
                          num_idxs=n_t * 128, num_idxs_reg=n_t * 128,
                          elem_size=64, transpose=False)

                  s_all = wpool.tile([128, T_b, 12], F32, tag="sall")
                  v_all = wpool.tile([128, T_b, 192], FP16, tag="vall")
                  agg_p = apool.tile([128, 196], F32, space="PSUM", tag="agg")

                  def hslice(t, l):
                      if t < T_lo:
                          reg, tt = hist_lo[t // CH], t % CH
                      else:
                          reg, tt = hist_hi[(t - T_lo) // CH], (t - T_lo) % CH
                      if l == 0:
                          return reg[0:64, 0, tt * 128:(tt + 1) * 128]
                      if l == 1:
                          return reg[64:128, 0, tt * 128:(tt + 1) * 128]
                      return reg[0:64, 1, tt * 128:(tt + 1) * 128]

                  if stage <= 2:
                      dbg2 = spool.tile([128, 64], F32, tag="dbg2")
                      nc.vector.tensor_copy(out=dbg2[:], in_=q_g[:, 0, :])
                      nc.sync.dma_start(out=out_d[b * 128:(b + 1) * 128, :], in_=dbg2[:])
                      continue
                  for t in range(T_b):
                      # one PSUM bank (512 f32) per matmul output: bank-aligned sub-writes
                      kv_p = kvpool.tile([128, 1536], F32, space="PSUM", tag="kv")
                      for l in range(3):
                          nc.tensor.matmul(
                              kv_p[:, l * 512:l * 512 + 128],
                              lhsT=hslice(t, l),
                              rhs=WkvT[64:128, :] if l == 1 else WkvT[0:64, :],
                              start=True, stop=True)
                      kk = kv_p[:].rearrange("p (l o) -> p l o", l=3)[:, :, 0:64]
                      vv = kv_p[:].rearrange("p (l o) -> p l o", l=3)[:, :, 64:128]
                      qk = spool.tile([128, 192], F32, tag="qk")
                      if "qk" in ablate:
                          continue
                      nc.vector.tensor_tensor(
                          out=qk[:].rearrange("p (l c) -> p l c", l=3),
                          in0=kk,
                          in1=q_g[:, t, :].unsqueeze(1).to_broadcast([128, 3, 64]),
                          op=mybir.AluOpType.mult)
                      nc.vector.tensor_reduce(
                          out=s_all[:, t, :],
                          in_=qk[:].rearrange("p (lh d) -> p lh d", d=16),
                          axis=mybir.AxisListType.X, op=mybir.AluOpType.add)
                      if "vcopy" not in ablate:
                          nc.scalar.copy(out=v_all[:, t, :], in_=vv)

                  if stage <= 3:
                      dbg = spool.tile([128, 64], F32, tag="dbg")
                      nc.vector.tensor_copy(out=dbg[:], in_=q_g[:, 0, :])
                      nc.sync.dma_start(out=out_d[b * 128:(b + 1) * 128, :], in_=dbg[:])
                      continue
                  u_all = wpool.tile([128, T_b, 12], F32, tag="uall")
                  nc.scalar.activation(out=u_all[:].rearrange("p t x -> p (t x)"),
                                       in_=s_all[:].rearrange("p t x -> p (t x)"),
                                       func=mybir.ActivationFunctionType.Exp)
                  u_lh = u_all[:].rearrange("p t (l h) -> p t h l", l=3, h=4)
                  U_all = spool.tile([128, T_b, 4], F32, tag="Uall")
                  nc.vector.tensor_reduce(out=U_all[:], in_=u_lh,
                                          axis=mybir.AxisListType.X, op=mybir.AluOpType.add)
                  m_all = spool.tile([128, T_b, 4], F32, tag="mall")
                  nc.vector.tensor_reduce(out=m_all[:], in_=u_lh,
                                          axis=mybir.AxisListType.X, op=mybir.AluOpType.max)
                  rU = spool.tile([128, T_b, 4], F32, tag="rU")
                  nc.vector.reciprocal(out=rU[:], in_=U_all[:])
                  f_all = spool.tile([128, T_b, 4], F32, tag="fall")
                  nc.vector.tensor_tensor(out=f_all[:], in0=m_all[:], in1=rU[:],
                                          op=mybir.AluOpType.mult)
                  w_all = wpool.tile([128, T_b, 12], FP16, tag="wall")
                  nc.vector.tensor_tensor(
                      out=w_all[:].rearrange("p t (l h) -> p t l h", l=3),
                      in0=u_all[:].rearrange("p t (l h) -> p t l h", l=3),
                      in1=f_all[:].unsqueeze(2).to_broadcast([128, T_b, 3, 4]),
                      op=mybir.AluOpType.mult)

                  if stage <= 4:
                      dbg3 = spool.tile([128, 64], F32, tag="dbg3")
                      nc.vector.tensor_copy(out=dbg3[:], in_=w_all[:, 0, 0:12].to_broadcast([128, 64]))
                      nc.sync.dma_start(out=out_d[b * 128:(b + 1) * 128, :], in_=dbg3[:])
                      continue
                  payload = wpool.tile([128, T_b, 196], FP16, tag="payload")
                  nc.vector.tensor_copy(out=payload[:, :, 192:196], in_=m_all[:])
                  for t in range(T_b):
                      nc.vector.tensor_tensor(
                          out=payload[:, t, 0:192].rearrange("p (l h d) -> p l h d", l=3, h=4),
                          in0=v_all[:, t, :].rearrange("p (l h d) -> p l h d", l=3, h=4),
                          in1=w_all[:, t, :].rearrange("p (l h) -> p l h", l=3)
                              .unsqueeze(-1).to_broadcast([128, 3, 4, 16]),
                          op=mybir.AluOpType.mult)
                      E_t = spool.tile([128, 128], FP16, tag="Et")
                      nc.vector.tensor_scalar(
                          out=E_t[:], in0=iota[:],
                          scalar1=dst_rel[:, b * T_b + t, None], scalar2=None,
                          op0=mybir.AluOpType.is_equal)
                      nc.tensor.matmul(agg_p[:], lhsT=E_t[:], rhs=payload[:, t, :],
                                       start=(t == 0), stop=(t == T_b - 1))

                  if stage <= 5:
                      dbg4 = spool.tile([128, 64], F32, tag="dbg4")
                      nc.vector.tensor_copy(out=dbg4[:], in_=agg_p[:, 0:64])
                      nc.sync.dma_start(out=out_d[b * 128:(b + 1) * 128, :], in_=dbg4[:])
                      continue
                  num = spool.tile([128, 64], F32, tag="num")
                  nc.vector.tensor_reduce(
                      out=num[:],
                      in_=agg_p[:, 0:192].rearrange("p (l hd) -> p hd l", l=3),
                      axis=mybir.AxisListType.X, op=mybir.AluOpType.add)
                  den = spool.tile([128, 4], F32, tag="den")
                  nc.vector.tensor_scalar(
                      out=den[:], in0=agg_p[:, 192:196], scalar1=1e-30, scalar2=None,
                      op0=mybir.AluOpType.add)
                  rden = spool.tile([128, 4], F32, tag="rden")
                  nc.vector.reciprocal(out=rden[:], in_=den[:])
                  aggv = spool.tile([128, 64], F32, tag="aggv")
                  nc.vector.tensor_tensor(
                      out=aggv[:].rearrange("p (h d) -> p h d", h=4),
                      in0=num[:].rearrange("p (h d) -> p h d", h=4),
                      in1=rden[:].unsqueeze(-1).to_broadcast([128, 4, 16]),
                      op=mybir.AluOpType.mult)
                  gate = spool.tile([128, 4], F32, tag="gate")
                  nc.vector.tensor_scalar(
                      out=gate[:], in0=agg_p[:, 192:196], scalar1=0.0, scalar2=None,
                      op0=mybir.AluOpType.is_gt)
                  bvg = spool.tile([128, 64], F32, tag="bvg")
                  nc.vector.tensor_tensor(
                      out=bvg[:].rearrange("p (h d) -> p h d", h=4),
                      in0=bv_rep[:].rearrange("p (h d) -> p h d", h=4),
                      in1=gate[:].unsqueeze(-1).to_broadcast([128, 4, 16]),
                      op=mybir.AluOpType.mult)
                  nc.vector.tensor_tensor(out=aggv[:], in0=aggv[:], in1=bvg[:],
                                          op=mybir.AluOpType.add)
                  aggT_p = ppool.tile([64, 128], F32, space="PSUM", tag="misc", name="aggT_p")
                  nc.tensor.transpose(out=aggT_p[:], in_=aggv[:], identity=ident[:])
                  aggT = spool.tile([64, 128], F32, tag="aggTs")
                  nc.vector.tensor_copy(out=aggT[:], in_=aggT_p[:])
                  o_p = ppool.tile([128, 64], F32, space="PSUM", tag="misc", name="o_p")
                  nc.tensor.matmul(o_p[:], lhsT=aggT[:], rhs=WoT[:], start=True, stop=True)
                  o_sb = spool.tile([128, 64], F32, tag="osb")
                  nc.vector.tensor_tensor(out=o_sb[:], in0=o_p[:], in1=bo_rep[:],
                                          op=mybir.AluOpType.add)
                  cur_blk2 = spool.tile([128, 64], F32, tag="curblk2")
                  nc.sync.dma_start(out=cur_blk2[:], in_=cur[b * 128:(b + 1) * 128, :])
                  nc.vector.tensor_tensor(out=o_sb[:], in0=o_sb[:], in1=cur_blk2[:],
                                          op=mybir.AluOpType.add)
                  nc.sync.dma_start(out=out_d[b * 128:(b + 1) * 128, :], in_=o_sb[:])

    return _finish(nc)


def _finish(nc):
    nc.compile()
    # insert_library_loads/insert_act_table_loads (run at the end of compile())
    # can reintroduce multi-wait instructions; re-split to satisfy the
    # 1-wait-per-instruction hardware constraint.
    nc.generate_event_semaphores()
    nc.codegen_inst_isa_subclasses()
    return nc


def assemble(results, params, n_src):
    """Gather per-core 'out' slices into the full [N, C] output."""
    npc = params["nodes_per_core"]
    outs = []
    for c, r in enumerate(results):
        nreal = min(npc, n_src - c * npc)
        outs.append(np.asarray(r["out"])[:nreal])
    return np.concatenate(outs, axis=0)


_CACHE = {}


def kernel(**inputs):
    import numpy as np
    from concourse.bass_utils import run_bass_kernel_spmd
    inputs = {k: np.asarray(v) for k, v in inputs.items()}
    in_maps, params = host_prep(inputs, ncores=8)
    key = (params["T_lo"], params["T_hi"], params["n_blocks"], params["n_src"])
    if key not in _CACHE:
        _CACHE[key] = build(params)
    nc = _CACHE[key]
    res = run_bass_kernel_spmd(nc, in_maps, core_ids=list(range(8)))
    return assemble(res.results, params, inputs["history"].shape[0]).astype(np.float32)



# revision 7
# speedup vs baseline: 527.6715x; 527.6715x over previous
"""Self-contained Trainium2 Bass kernel for the DNAConv GNN message-passing problem.

kernel(**inputs) takes the FULL unsharded inputs and returns the FULL [50000, 64]
float32 output. Edges are sharded across 8 NeuronCores by destination-node range
(6250 nodes/core), so each core owns its output rows and no collectives are needed.
Per core: history rows are fetched per edge with SWDGE transpose-gathers (fp16),
K/V are computed per edge on the PE (fp16 matmuls), token softmax + edge softmax
are fused into an exp-normalize form on DVE/ACT, and per-destination aggregation
is a one-hot segment-sum matmul accumulated in PSUM (128-node blocks).
"""
"""GNN DNAConv kernel for TRN2: builder + host prep.

Per-core (dst-sharded) algorithm:
  out[n] = (Num[n]/Den[n] + bv) @ Wo.T + bo + current[n]
  Num[n,h,d] = sum_{e->n} sum_l w[e,l,h] * v[e,l,h,d]
  Den[n,h]   = sum_{e->n} m[e,h]
  w[e,l,h] = u[e,l,h] * m[e,h] / U[e,h];  u = exp(s), U = sum_l u, m = max_l u
  s[e,l,h] = sum_d q[dst,h,d]*k[src,l,h,d] / 4   (the 1/4 is folded into q)
  (bk provably has no effect; bq/bv/bo handled at node level.)
"""
import os
import tempfile

import numpy as np

try:  # persistent jax compilation cache: avoid recompiling identical programs
    import jax

    jax.config.update(
        "jax_compilation_cache_dir",
        os.path.join(tempfile.gettempdir(), "jax_bass_cache"),
    )
    jax.config.update("jax_persistent_cache_min_compile_time_secs", 0.0)
    jax.config.update("jax_persistent_cache_min_entry_size_bytes", -1)
except Exception:
    pass

import concourse.bacc as bacc
import concourse.tile as tile
from concourse import bass, mybir
from concourse.masks import make_identity

FP16 = mybir.dt.float16
F32 = mybir.dt.float32
I16 = mybir.dt.int16


def wrap16_rep(idx):
    """SWDGE idx layout: [128, n/16], elem j at [j%16, j//16], replicated x8."""
    idx = np.asarray(idx, np.int16)
    n = idx.shape[0]
    assert n % 16 == 0
    w = idx.reshape(n // 16, 16).T
    return np.tile(w, (8, 1)).copy()


def host_prep(inputs, ncores=8):
    hist = np.asarray(inputs["history"], np.float32)
    ei = np.asarray(inputs["edge_index"])
    n_src, L, C = hist.shape
    Wq = np.asarray(inputs["Wq"], np.float32); bq = np.asarray(inputs["bq"], np.float32)
    Wk = np.asarray(inputs["Wk"], np.float32)
    Wv = np.asarray(inputs["Wv"], np.float32); bv = np.asarray(inputs["bv"], np.float32)
    Wo = np.asarray(inputs["Wo"], np.float32); bo = np.asarray(inputs["bo"], np.float32)
    row, col = ei[0].astype(np.int64), ei[1].astype(np.int64)

    nodes_per_core = (n_src + ncores - 1) // ncores
    nblk = (nodes_per_core + 127) // 128
    nloc = nblk * 128
    src_split = ((n_src + 1) // 2 + 127) // 128 * 128
    if src_split >= n_src:
        src_split = n_src // 2

    hist_sw = np.zeros((n_src, 256), np.float16)
    hist_sw[:, :L * C] = hist.reshape(n_src, L * C).astype(np.float16)

    order = np.argsort(col, kind="stable")
    row_s, col_s = row[order], col[order]
    core_of = col_s // nodes_per_core
    loc = col_s % nodes_per_core
    blk = loc // 128
    lo_side = row_s < src_split

    cnt = np.zeros((ncores, nblk, 2), np.int64)
    for c in range(ncores):
        mc = core_of == c
        bb = blk[mc]
        ll = lo_side[mc]
        for b in range(nblk):
            mb = bb == b
            cnt[c, b, 0] = (mb & ll).sum()
            cnt[c, b, 1] = (mb & ~ll).sum()
    T_lo = max(1, int(np.ceil(cnt[:, :, 0].max() / 128)))
    T_hi = max(1, int(np.ceil(cnt[:, :, 1].max() / 128)))
    T_b = T_lo + T_hi
    CH = 6  # tiles per gather chunk (<= 1000 descriptor HW limit / 128)

    WkvT1 = np.concatenate([Wk.T, Wv.T], axis=1).astype(np.float16)
    WkvT = np.concatenate([WkvT1, WkvT1], axis=0)  # duplicated in both partition halves
    WqT = (Wq.T * (1.0 / np.sqrt(C // 4))).astype(np.float32)
    WoT = Wo.T.astype(np.float32)
    bq_rep = np.tile(bq * (1.0 / np.sqrt(C // 4)), (128, 1)).astype(np.float32)
    bv_rep = np.tile(bv, (128, 1)).astype(np.float32)
    bo_rep = np.tile(bo, (128, 1)).astype(np.float32)
    iota128 = np.tile(np.arange(128, dtype=np.float16), (128, 1))
    cur = hist[:, -1]

    in_maps = []
    for c in range(ncores):
        mc = core_of == c
        hist_idx_lo = np.zeros((nblk, 128, T_lo * 8), np.int16)
        hist_idx_hi = np.zeros((nblk, 128, T_hi * 8), np.int16)
        q_idx = np.zeros((nblk, 128, T_b * 8), np.int16)
        dst_rel = np.full((128, nblk * T_b), -1.0, np.float32)
        for b in range(nblk):
            mb = mc & (blk == b)
            r_lo, d_lo = row_s[mb & lo_side], loc[mb & lo_side]
            r_hi, d_hi = row_s[mb & ~lo_side], loc[mb & ~lo_side]
            n_lo, n_hi = len(r_lo), len(r_hi)
            assert n_lo <= T_lo * 128 and n_hi <= T_hi * 128
            slo = np.zeros(T_lo * 128, np.int64); slo[:n_lo] = r_lo
            shi = np.zeros(T_hi * 128, np.int64); shi[:n_hi] = r_hi - src_split
            hist_idx_lo[b] = wrap16_rep(slo)
            hist_idx_hi[b] = wrap16_rep(shi)
            qi = np.full(T_b * 128, 127, np.int64)
            dr = np.full(T_b * 128, -1.0, np.float32)
            qi[:n_lo] = d_lo % 128; dr[:n_lo] = d_lo % 128
            qi[T_lo * 128:T_lo * 128 + n_hi] = d_hi % 128
            dr[T_lo * 128:T_lo * 128 + n_hi] = d_hi % 128
            q_idx[b] = wrap16_rep(qi)
            dst_rel[:, b * T_b:(b + 1) * T_b] = dr.reshape(T_b, 128).T
        cur_c = np.zeros((nloc, C), np.float32)
        nreal = min(nodes_per_core, n_src - c * nodes_per_core)
        cur_c[:nreal] = cur[c * nodes_per_core:c * nodes_per_core + nreal]
        in_maps.append({
            "hist_sw": hist_sw,
            "cur": cur_c,
            "hist_idx_lo": hist_idx_lo, "hist_idx_hi": hist_idx_hi,
            "q_idx": q_idx, "dst_rel": dst_rel,
            "WkvT": np.asarray(WkvT), "WqT": WqT, "WoT": WoT,
            "bq_rep": bq_rep, "bv_rep": bv_rep, "bo_rep": bo_rep,
            "iota128": iota128,
        })
    params = dict(T_lo=T_lo, T_hi=T_hi, n_blocks=nblk, n_src=n_src, nloc=nloc,
                  src_split=src_split, nodes_per_core=nodes_per_core, ncores=ncores)
    return in_maps, params


def build(params, stage=99, reps=1, ablate=()):
    T_lo, T_hi = params["T_lo"], params["T_hi"]
    T_b = T_lo + T_hi
    NB = params["n_blocks"]
    NSRC = params["n_src"]
    NLOC = params["nloc"]
    SPLIT = params["src_split"]

    nc = bacc.Bacc(None, target_bir_lowering=False)
    hist_sw = nc.declare_dram_parameter("hist_sw", [NSRC, 256], FP16, isOutput=False)
    cur = nc.declare_dram_parameter("cur", [NLOC, 64], F32, isOutput=False)
    hidx_lo = nc.declare_dram_parameter("hist_idx_lo", [NB, 128, T_lo * 8], I16, isOutput=False)
    hidx_hi = nc.declare_dram_parameter("hist_idx_hi", [NB, 128, T_hi * 8], I16, isOutput=False)
    qidx = nc.declare_dram_parameter("q_idx", [NB, 128, T_b * 8], I16, isOutput=False)
    dst_rel_d = nc.declare_dram_parameter("dst_rel", [128, NB * T_b], F32, isOutput=False)
    WkvT_d = nc.declare_dram_parameter("WkvT", [128, 128], FP16, isOutput=False)
    WqT_d = nc.declare_dram_parameter("WqT", [64, 64], F32, isOutput=False)
    WoT_d = nc.declare_dram_parameter("WoT", [64, 64], F32, isOutput=False)
    bq_rep_d = nc.declare_dram_parameter("bq_rep", [128, 64], F32, isOutput=False)
    bv_rep_d = nc.declare_dram_parameter("bv_rep", [128, 64], F32, isOutput=False)
    bo_rep_d = nc.declare_dram_parameter("bo_rep", [128, 64], F32, isOutput=False)
    iota_d = nc.declare_dram_parameter("iota128", [128, 128], FP16, isOutput=False)
    out_d = nc.declare_dram_parameter("out", [NLOC, 64], F32, isOutput=True)
    q_dram = nc.dram_tensor("q_table", [NLOC, 64], F32)

    with tile.TileContext(nc) as tc:
        with (
            tc.tile_pool(name="const", bufs=1) as cpool,
            tc.tile_pool(name="meta", bufs=1) as mpool,
            tc.tile_pool(name="idxp", bufs=2) as ipool,
            tc.tile_pool(name="work", bufs=2) as wpool,
            tc.tile_pool(name="small", bufs=3) as spool,
            tc.tile_pool(name="psum", bufs=1, space="PSUM") as ppool,
            tc.tile_pool(name="aggpsum", bufs=1, space="PSUM") as apool,
            tc.tile_pool(name="kvpsum", bufs=2, space="PSUM") as kvpool,
        ):
            WkvT = cpool.tile([128, 128], FP16)
            nc.sync.dma_start(out=WkvT[:], in_=WkvT_d[:])
            WqT = cpool.tile([64, 64], F32)
            nc.sync.dma_start(out=WqT[:], in_=WqT_d[:])
            WoT = cpool.tile([64, 64], F32)
            nc.sync.dma_start(out=WoT[:], in_=WoT_d[:])
            bq_rep = cpool.tile([128, 64], F32)
            nc.sync.dma_start(out=bq_rep[:], in_=bq_rep_d[:])
            bv_rep = cpool.tile([128, 64], F32)
            nc.sync.dma_start(out=bv_rep[:], in_=bv_rep_d[:])
            bo_rep = cpool.tile([128, 64], F32)
            nc.sync.dma_start(out=bo_rep[:], in_=bo_rep_d[:])
            iota = cpool.tile([128, 128], FP16)
            nc.sync.dma_start(out=iota[:], in_=iota_d[:])
            ident = cpool.tile([128, 128], F32)
            make_identity(nc, ident[:])
            dst_rel = mpool.tile([128, NB * T_b], F32)
            nc.sync.dma_start(out=dst_rel[:], in_=dst_rel_d[:])

            for _rep in range(reps):
              # ---- Phase 1: q table: q = (cur @ Wq.T + bq)/sqrt(D) ----
              for b in range(NB):
                  cur_blk = spool.tile([128, 64], F32, tag="curblk")
                  nc.sync.dma_start(out=cur_blk[:], in_=cur[b * 128:(b + 1) * 128, :])
                  curT_p = ppool.tile([64, 128], F32, space="PSUM", tag="misc")
                  nc.tensor.transpose(out=curT_p[:], in_=cur_blk[:], identity=ident[:])
                  curT = spool.tile([64, 128], F32, tag="curT")
                  nc.vector.tensor_copy(out=curT[:], in_=curT_p[:])
                  q_p = ppool.tile([128, 64], F32, space="PSUM", tag="misc", name="q_p")
                  nc.tensor.matmul(q_p[:], lhsT=curT[:], rhs=WqT[:], start=True, stop=True)
                  q_sb = spool.tile([128, 64], F32, tag="qsb")
                  nc.vector.tensor_tensor(out=q_sb[:], in0=q_p[:], in1=bq_rep[:],
                                          op=mybir.AluOpType.add)
                  nc.sync.dma_start(out=q_dram[b * 128:(b + 1) * 128, :], in_=q_sb[:])
                  if stage <= 1:
                      nc.sync.dma_start(out=out_d[b * 128:(b + 1) * 128, :], in_=q_sb[:])

              # ---- Phase 2: per-block edge processing ----
              for b in range(NB if stage >= 2 else 0):
                  il = ipool.tile([128, T_lo * 8], I16, tag="il")
                  nc.sync.dma_start(out=il[:], in_=hidx_lo[b])
                  ih = ipool.tile([128, T_hi * 8], I16, tag="ih")
                  nc.sync.dma_start(out=ih[:], in_=hidx_hi[b])
                  iq = ipool.tile([128, T_b * 8], I16, tag="iq")
                  nc.sync.dma_start(out=iq[:], in_=qidx[b])

                  CH = 1 if "ch1" in ablate else 6
                  def hgather(hc, tbl, idx_ap, n_t):
                      if "histsmall" in ablate:
                          out = hc[:].rearrange("p a b -> p (a b)")[:, 0:256]\
                                     .rearrange("p (a b) -> p a b", a=2)
                          nc.gpsimd.dma_gather(
                              out_ap=out, in_ap=tbl, idxs_ap=idx_ap[:, 0:8],
                              num_idxs=128, num_idxs_reg=128,
                              elem_size=256, transpose=True)
                      else:
                          nc.gpsimd.dma_gather(
                              out_ap=hc[:], in_ap=tbl, idxs_ap=idx_ap,
                              num_idxs=n_t * 128, num_idxs_reg=n_t * 128,
                              elem_size=256, transpose=True)
                  hist_lo = []
                  for c in range(0, T_lo, CH):
                      n_t = min(CH, T_lo - c)
                      hc = wpool.tile([128, 2, n_t * 128], FP16, tag=f"histlo{c}",
                                      name=f"histlo_{b}_{c}")
                      hgather(hc, hist_sw[0:SPLIT, :], il[:, c * 8:(c + n_t) * 8], n_t)
                      hist_lo.append(hc)
                  hist_hi = []
                  for c in range(0, T_hi, CH):
                      n_t = min(CH, T_hi - c)
                      hc = wpool.tile([128, 2, n_t * 128], FP16, tag=f"histhi{c}",
                                      name=f"histhi_{b}_{c}")
                      hgather(hc, hist_sw[SPLIT:NSRC, :], ih[:, c * 8:(c + n_t) * 8], n_t)
                      hist_hi.append(hc)
                  q_g = wpool.tile([128, T_b, 64], F32, tag="qg")
                  for c in range(0, T_b, CH):
                      n_t = min(CH, T_b - c)
                      ng = 128 if "qsmall" in ablate else n_t * 128
                      nc.gpsimd.dma_gather(
                          out_ap=q_g[:, c:c + ng // 128, :],
                          in_ap=q_dram[b * 128:(b + 1) * 128, :],
                          idxs_ap=iq[:, c * 8:c * 8 + ng // 16],
                          num_idxs=ng, num_idxs_reg=ng,
                          elem_size=64, transpose=False)

                  s_all = wpool.tile([128, T_b, 12], F32, tag="sall")
                  v_all = wpool.tile([128, T_b, 192], FP16, tag="vall")
                  agg_p = apool.tile([128, 196], F32, space="PSUM", tag="agg")

                  def hslice(t, l):
                      if t < T_lo:
                          reg, tt = hist_lo[t // CH], t % CH
                      else:
                          reg, tt = hist_hi[(t - T_lo) // CH], (t - T_lo) % CH
                      if l == 0:
                          return reg[0:64, 0, tt * 128:(tt + 1) * 128]
                      if l == 1:
                          return reg[64:128, 0, tt * 128:(tt + 1) * 128]
                      return reg[0:64, 1, tt * 128:(tt + 1) * 128]

                  if stage <= 2:
                      dbg2 = spool.tile([128, 64], F32, tag="dbg2")
                      nc.vector.tensor_copy(out=dbg2[:], in_=q_g[:, 0, :])
                      nc.sync.dma_start(out=out_d[b * 128:(b + 1) * 128, :], in_=dbg2[:])
                      continue
                  for t in range(T_b):
                      # one PSUM bank (512 f32) per matmul output: bank-aligned sub-writes
                      kv_p = kvpool.tile([128, 1536], F32, space="PSUM", tag="kv")
                      for l in range(3):
                          nc.tensor.matmul(
                              kv_p[:, l * 512:l * 512 + 128],
                              lhsT=hslice(t, l),
                              rhs=WkvT[64:128, :] if l == 1 else WkvT[0:64, :],
                              start=True, stop=True)
                      kk = kv_p[:].rearrange("p (l o) -> p l o", l=3)[:, :, 0:64]
                      vv = kv_p[:].rearrange("p (l o) -> p l o", l=3)[:, :, 64:128]
                      qk = spool.tile([128, 192], F32, tag="qk")
                      if "qk" in ablate:
                          continue
                      nc.vector.tensor_tensor(
                          out=qk[:].rearrange("p (l c) -> p l c", l=3),
                          in0=kk,
                          in1=q_g[:, t, :].unsqueeze(1).to_broadcast([128, 3, 64]),
                          op=mybir.AluOpType.mult)
                      nc.vector.tensor_reduce(
                          out=s_all[:, t, :],
                          in_=qk[:].rearrange("p (lh d) -> p lh d", d=16),
                          axis=mybir.AxisListType.X, op=mybir.AluOpType.add)
                      if "vcopy" not in ablate:
                          nc.scalar.copy(out=v_all[:, t, :], in_=vv)

                  if stage <= 3:
                      dbg = spool.tile([128, 64], F32, tag="dbg")
                      nc.vector.tensor_copy(out=dbg[:], in_=q_g[:, 0, :])
                      nc.sync.dma_start(out=out_d[b * 128:(b + 1) * 128, :], in_=dbg[:])
                      continue
                  u_all = wpool.tile([128, T_b, 12], F32, tag="uall")
                  m_all = spool.tile([128, T_b, 4], F32, tag="mall")
                  w_all = wpool.tile([128, T_b, 12], FP16, tag="wall")
                  if "soft" in ablate:
                      nc.vector.memset(w_all[:], 0.5)
                      nc.vector.memset(m_all[:], 0.5)
                  else:
                    nc.scalar.activation(out=u_all[:].rearrange("p t x -> p (t x)"),
                                         in_=s_all[:].rearrange("p t x -> p (t x)"),
                                         func=mybir.ActivationFunctionType.Exp)
                    u_lh = u_all[:].rearrange("p t (l h) -> p t h l", l=3, h=4)
                    U_all = spool.tile([128, T_b, 4], F32, tag="Uall")
                    nc.vector.tensor_reduce(out=U_all[:], in_=u_lh,
                                            axis=mybir.AxisListType.X, op=mybir.AluOpType.add)
                    nc.vector.tensor_reduce(out=m_all[:], in_=u_lh,
                                            axis=mybir.AxisListType.X, op=mybir.AluOpType.max)
                    rU = spool.tile([128, T_b, 4], F32, tag="rU")
                    nc.vector.reciprocal(out=rU[:], in_=U_all[:])
                    f_all = spool.tile([128, T_b, 4], F32, tag="fall")
                    nc.vector.tensor_tensor(out=f_all[:], in0=m_all[:], in1=rU[:],
                                            op=mybir.AluOpType.mult)
                    nc.vector.tensor_tensor(
                        out=w_all[:].rearrange("p t (l h) -> p t l h", l=3),
                        in0=u_all[:].rearrange("p t (l h) -> p t l h", l=3),
                        in1=f_all[:].unsqueeze(2).to_broadcast([128, T_b, 3, 4]),
                        op=mybir.AluOpType.mult)

                  if stage <= 4:
                      dbg3 = spool.tile([128, 64], F32, tag="dbg3")
                      nc.vector.tensor_copy(out=dbg3[:], in_=u_all[:, 0:5, :].rearrange("p a b -> p (a b)")[:, 0:64])
                      nc.sync.dma_start(out=out_d[b * 128:(b + 1) * 128, :], in_=dbg3[:])
                      continue
                  payload = wpool.tile([128, T_b, 196], FP16, tag="payload")
                  nc.vector.tensor_copy(out=payload[:, :, 192:196], in_=m_all[:])
                  for t in range(T_b):
                      if "pay" not in ablate:
                          nc.vector.tensor_tensor(
                              out=payload[:, t, 0:192].rearrange("p (l h d) -> p l h d", l=3, h=4),
                              in0=v_all[:, t, :].rearrange("p (l h d) -> p l h d", l=3, h=4),
                              in1=w_all[:, t, :].rearrange("p (l h) -> p l h", l=3)
                                  .unsqueeze(-1).to_broadcast([128, 3, 4, 16]),
                              op=mybir.AluOpType.mult)
                      if "eq" in ablate:
                          E_t = iota
                      else:
                          E_t = spool.tile([128, 128], FP16, tag="Et")
                          nc.vector.tensor_scalar(
                              out=E_t[:], in0=iota[:],
                              scalar1=dst_rel[:, b * T_b + t, None], scalar2=None,
                              op0=mybir.AluOpType.is_equal)
                      if "aggmm" in ablate:
                          if t == 0:
                              nc.tensor.matmul(agg_p[:], lhsT=E_t[:], rhs=payload[:, t, :],
                                               start=True, stop=True)
                      else:
                          nc.tensor.matmul(agg_p[:], lhsT=E_t[:], rhs=payload[:, t, :],
                                           start=(t == 0), stop=(t == T_b - 1))

                  if stage <= 5:
                      dbg4 = spool.tile([128, 64], F32, tag="dbg4")
                      nc.vector.tensor_copy(out=dbg4[:], in_=agg_p[:, 0:64])
                      nc.sync.dma_start(out=out_d[b * 128:(b + 1) * 128, :], in_=dbg4[:])
                      continue
                  num = spool.tile([128, 64], F32, tag="num")
                  nc.vector.tensor_reduce(
                      out=num[:],
                      in_=agg_p[:, 0:192].rearrange("p (l hd) -> p hd l", l=3),
                      axis=mybir.AxisListType.X, op=mybir.AluOpType.add)
                  den = spool.tile([128, 4], F32, tag="den")
                  nc.vector.tensor_scalar(
                      out=den[:], in0=agg_p[:, 192:196], scalar1=1e-30, scalar2=None,
                      op0=mybir.AluOpType.add)
                  rden = spool.tile([128, 4], F32, tag="rden")
                  nc.vector.reciprocal(out=rden[:], in_=den[:])
                  aggv = spool.tile([128, 64], F32, tag="aggv")
                  nc.vector.tensor_tensor(
                      out=aggv[:].rearrange("p (h d) -> p h d", h=4),
                      in0=num[:].rearrange("p (h d) -> p h d", h=4),
                      in1=rden[:].unsqueeze(-1).to_broadcast([128, 4, 16]),
                      op=mybir.AluOpType.mult)
                  gate = spool.tile([128, 4], F32, tag="gate")
                  nc.vector.tensor_scalar(
                      out=gate[:], in0=agg_p[:, 192:196], scalar1=0.0, scalar2=None,
                      op0=mybir.AluOpType.is_gt)
                  bvg = spool.tile([128, 64], F32, tag="bvg")
                  nc.vector.tensor_tensor(
                      out=bvg[:].rearrange("p (h d) -> p h d", h=4),
                      in0=bv_rep[:].rearrange("p (h d) -> p h d", h=4),
                      in1=gate[:].unsqueeze(-1).to_broadcast([128, 4, 16]),
                      op=mybir.AluOpType.mult)
                  nc.vector.tensor_tensor(out=aggv[:], in0=aggv[:], in1=bvg[:],
                                          op=mybir.AluOpType.add)
                  aggT_p = ppool.tile([64, 128], F32, space="PSUM", tag="misc", name="aggT_p")
                  nc.tensor.transpose(out=aggT_p[:], in_=aggv[:], identity=ident[:])
                  aggT = spool.tile([64, 128], F32, tag="aggTs")
                  nc.vector.tensor_copy(out=aggT[:], in_=aggT_p[:])
                  o_p = ppool.tile([128, 64], F32, space="PSUM", tag="misc", name="o_p")
                  nc.tensor.matmul(o_p[:], lhsT=aggT[:], rhs=WoT[:], start=True, stop=True)
                  o_sb = spool.tile([128, 64], F32, tag="osb")
                  nc.vector.tensor_tensor(out=o_sb[:], in0=o_p[:], in1=bo_rep[:],
                                          op=mybir.AluOpType.add)
                  cur_blk2 = spool.tile([128, 64], F32, tag="curblk2")
                  nc.sync.dma_start(out=cur_blk2[:], in_=cur[b * 128:(b + 1) * 128, :])
                  nc.vector.tensor_tensor(out=o_sb[:], in0=o_sb[:], in1=cur_blk2[:],
                                          op=mybir.AluOpType.add)
                  nc.sync.dma_start(out=out_d[b * 128:(b + 1) * 128, :], in_=o_sb[:])

    return _finish(nc)


def _finish(nc):
    nc.compile()
    # insert_library_loads/insert_act_table_loads (run at the end of compile())
    # can reintroduce multi-wait instructions; re-split to satisfy the
    # 1-wait-per-instruction hardware constraint.
    nc.generate_event_semaphores()
    nc.codegen_inst_isa_subclasses()
    return nc


def assemble(results, params, n_src):
    """Gather per-core 'out' slices into the full [N, C] output."""
    npc = params["nodes_per_core"]
    outs = []
    for c, r in enumerate(results):
        nreal = min(npc, n_src - c * npc)
        outs.append(np.asarray(r["out"])[:nreal])
    return np.concatenate(outs, axis=0)


_CACHE = {}


def kernel(**inputs):
    import numpy as np
    from concourse.bass_utils import run_bass_kernel_spmd
    inputs = {k: np.asarray(v) for k, v in inputs.items()}
    in_maps, params = host_prep(inputs, ncores=8)
    key = (params["T_lo"], params["T_hi"], params["n_blocks"], params["n_src"])
    if key not in _CACHE:
        _CACHE[key] = build(params)
    nc = _CACHE[key]
    res = run_bass_kernel_spmd(nc, in_maps, core_ids=list(range(8)))
    return assemble(res.results, params, inputs["history"].shape[0]).astype(np.float32)

